# revision 1
# baseline (speedup 1.0000x reference)
"""Trainium2 Bass kernel for nn_MetapathRecommender, v2.

Shapes (hardcoded): B=1024, C=8192, P=3, E=64, M=128, H=16, K=8, 8 cores.

Design vs v1 baseline:
  - pc = silu(ce @ mpk + b) and ncT (l2-normalized cards) are computed on the
    HOST in f64/f32 (they are small weight transforms); the device no longer
    runs the fp32 pc matmul phase or the normalize phase.
  - The dominant stream pcmT[p] = pc[p] @ metapaths[p][:, dshard] runs as a
    SINGLE fp16-lhsT pass (no hi+lo), with metapaths streamed as fp8-e3m4
    (half the HBM bytes of fp16; e3m4 behaves like 6-bit fixed point on
    uniform [0,1) data).  Optionally upconverted to fp16 on-chip if the PE
    does not accept mixed fp16 x fp8 operands.
  - The coherent part of the pc-quantization error (which the all-positive
    metapaths/pools sums amplify ~C^1 instead of ~sqrt(C)) is removed with an
    exact host-side rank-1 correction: err_x ~= Sdelta[m] * (pools @ mean_c
    mp)[b], added to x after the ReduceScatter.
  - Metapaths are host-repacked into stream order so every DMA is one fully
    contiguous 512KB block.

Per-core stream: (P, C, D=1024) e3m4 through the big matmul, partial
xT[p] = pcm[p].T @ poolsT, per-p ReduceScatter of x (overlapped), then the
attention tail + cosine head for this core's B/8 = 128 batch rows.
"""

import sys
from contextlib import ExitStack

sys.path.insert(0, "/opt/trn_rl_repo")

import numpy as np
import ml_dtypes

import concourse.bass as bass
import concourse.tile as tile
from concourse import mybir
from concourse.bass_utils import run_bass_kernel_spmd

F16 = mybir.dt.float16
F32 = mybir.dt.float32
F8E3 = mybir.dt.float8e3
F8E4 = mybir.dt.float8e4
AF = mybir.ActivationFunctionType
ALU = mybir.AluOpType

B, C, P, E, M, H, K = 1024, 8192, 3, 64, 128, 16, 8
NCORES = 8
D = C // NCORES          # 1024: d-shard width per core
NB = B // NCORES         # 128: batch rows per core for the tail
NCT = C // 128           # 64 c-blocks of 128
NG = NCT // 4            # 16 stream DMA groups (4 blocks = 512KB each)
NDT = D // 128           # 8 d tiles
EPS = 1e-12
INV_SQRT_K = 1.0 / float(np.sqrt(np.float32(K)))

# --- config ---
MP_DT = "e3"             # metapath stream dtype: "e3" | "e4" | "f16"
CONVERT = False          # upconvert mp tiles to fp16 on-chip before matmul
RS_F32 = False           # ReduceScatter x in f32 (False: fp16)
USE_A2A = False          # AllToAll + local sum instead of ReduceScatter

_MPD = {"e3": F8E3, "e4": F8E4, "f16": F16}
_MPNP = {"e3": ml_dtypes.float8_e3m4, "e4": ml_dtypes.float8_e4m3,
         "f16": np.float16}

_CACHE = {}


def _split_multi_waits(nc, cap=1):
    """Walrus in this container only accepts `cap` sync-waits per instruction.

    Move extra waits onto freshly inserted NoOps immediately preceding the
    instruction on the same engine."""
    f = nc.m.functions[0]
    nid = 0
    for blk in f.blocks:
        il = blk.instructions
        i = 0
        while i < len(il):
            inst = il[i]
            si = inst.sync_info
            if si is not None and len(si.on_wait) > cap:
                waits = list(si.on_wait)
                extra, keep = waits[:-cap], waits[-cap:]
                for w in extra:
                    nop = mybir.InstNoOp(
                        name=f"I-wsplit-{nid}", engine=inst.engine,
                        sync_info=mybir.SyncInfo(on_wait=[w], on_update=[]))
                    nid += 1
                    il.insert(i, nop)
                    i += 1
                inst.sync_info = mybir.SyncInfo(
                    on_wait=keep, on_update=list(si.on_update))
            i += 1
    return nid


def build_kernel(no_cc=False, repeat=1, mp_dt=None, convert=None, ablate=None,
                 a2a=None):
    mp_dt = MP_DT if mp_dt is None else mp_dt
    convert = CONVERT if convert is None else convert
    a2a = USE_A2A if a2a is None else a2a
    mpd = _MPD[mp_dt]
    nc = bass.Bass(num_devices=NCORES)
    rs_dt = F32 if RS_F32 else F16

    # ---- kernel I/O (per-core shards / replicated small tensors) ----
    mp_d = nc.dram_tensor("mp_stream", [P, NG, 128, 4 * D], mpd,
                          kind="ExternalInput")
    poolsT_d = nc.dram_tensor("poolsT_shard", [D, B], F16, kind="ExternalInput")
    pcT_d = nc.dram_tensor("pcT16", [128, NCT * P * M], F16, kind="ExternalInput")
    ncT_d = nc.dram_tensor("ncT", [E + 1, C], F16, kind="ExternalInput")
    corr_d = nc.dram_tensor("corr", [P, M, NB], F32, kind="ExternalInput")
    wq_d = nc.dram_tensor("wq", [M, H * K], F32, kind="ExternalInput")
    wk_d = nc.dram_tensor("wk", [M, H * K], F32, kind="ExternalInput")
    wv_d = nc.dram_tensor("wv", [M, H * K], F32, kind="ExternalInput")
    wo_d = nc.dram_tensor("wo", [H * K, M], F32, kind="ExternalInput")
    bq_d = nc.dram_tensor("bq_bc", [NB, H * K], F32, kind="ExternalInput")
    bk_d = nc.dram_tensor("bk_bc", [NB, H * K], F32, kind="ExternalInput")
    bv_d = nc.dram_tensor("bv_bc", [NB, H * K], F32, kind="ExternalInput")
    bo3_d = nc.dram_tensor("bo3_col", [M, 1], F32, kind="ExternalInput")
    pk_d = nc.dram_tensor("pool_kernel", [M, E], F32, kind="ExternalInput")
    pb_d = nc.dram_tensor("pool_bias_bc", [NB, E], F32, kind="ExternalInput")
    ident_h_d = nc.dram_tensor("ident_h", [128, 128], F16, kind="ExternalInput")
    ident_f_d = nc.dram_tensor("ident_f", [128, 128], F32, kind="ExternalInput")

    out_d = nc.dram_tensor("out", [NB, C], F16, kind="ExternalOutput")

    with ExitStack() as ctx:
        tc = ctx.enter_context(tile.TileContext(nc, num_cores=NCORES))

        const = ctx.enter_context(tc.tile_pool(name="const", bufs=1))
        dram = ctx.enter_context(tc.tile_pool(name="dram", bufs=1, space="DRAM"))

        # ---------- load constants / replicated weights ----------
        poolsT_sb = const.tile([128, NDT, B], F16)  # (d%128, dtile, b)
        nc.scalar.dma_start(poolsT_sb[:], poolsT_d.ap().rearrange("(t p) b -> p t b", p=128))
        pcT_sb = const.tile([128, P, NCT, M], F16, name="pcT")
        for p in range(P):
            nc.scalar.dma_start(
                pcT_sb[:, p, :, :].rearrange("q t m -> q (t m)"),
                pcT_d[:, p * NCT * M:(p + 1) * NCT * M])
        ncT_sb = const.tile([E + 1, C], F16)
        nc.scalar.dma_start(ncT_sb[:], ncT_d[:, :])
        # npT row E is the constant 0.5 that turns npT.T @ ncT_aug into
        # (cos + 1)/2 directly (ncT_aug's last row is all ones)
        npT = const.tile([E + 1, NB], F16, name="npT")
        nc.vector.memset(npT[E:E + 1, :], 0.5)
        corr_sb = [const.tile([M, NB], F32, name=f"corr{p}") for p in range(P)]
        for p in range(P):
            nc.scalar.dma_start(corr_sb[p][:], corr_d[p, :, :])
        wq_sb = const.tile([M, H * K], F32)
        nc.scalar.dma_start(wq_sb[:], wq_d[:, :])
        wk_sb = const.tile([M, H * K], F32)
        nc.scalar.dma_start(wk_sb[:], wk_d[:, :])
        wv_sb = const.tile([M, H * K], F32)
        nc.scalar.dma_start(wv_sb[:], wv_d[:, :])
        wo_sb = const.tile([H * K, M], F32)
        nc.scalar.dma_start(wo_sb[:], wo_d[:, :])
        bq_sb = const.tile([NB, H * K], F32)
        nc.scalar.dma_start(bq_sb[:], bq_d[:, :])
        bk_sb = const.tile([NB, H * K], F32)
        nc.scalar.dma_start(bk_sb[:], bk_d[:, :])
        bv_sb = const.tile([NB, H * K], F32)
        nc.scalar.dma_start(bv_sb[:], bv_d[:, :])
        bo3_sb = const.tile([M, 1], F32)
        nc.scalar.dma_start(bo3_sb[:], bo3_d[:, :])
        pk_sb = const.tile([M, E], F32)
        nc.scalar.dma_start(pk_sb[:], pk_d[:, :])
        pb_sb = const.tile([NB, E], F32)
        nc.scalar.dma_start(pb_sb[:], pb_d[:, :])
        ident_h = const.tile([128, 128], F16)
        nc.scalar.dma_start(ident_h[:], ident_h_d[:, :])
        ident_f = const.tile([128, 128], F32)
        nc.scalar.dma_start(ident_f[:], ident_f_d[:, :])
        eps_sb = const.tile([128, 1], F32)
        nc.vector.memset(eps_sb[:], 4.0 * EPS)

        for _rep in range(repeat):
            # stream-phase psum pools (released before the tail)
            stream_psum_ctx = ExitStack()
            mm_psum = stream_psum_ctx.enter_context(
                tc.tile_pool(name="mm_psum", bufs=2, space="PSUM"))      # 4 banks
            trx_psum = stream_psum_ctx.enter_context(
                tc.tile_pool(name="trx_psum", bufs=1, space="PSUM"))     # 3 banks
            stream_sbuf_ctx = ExitStack()
            mp_pool = stream_sbuf_ctx.enter_context(tc.tile_pool(name="mp", bufs=12))
            cvt_pool = (stream_sbuf_ctx.enter_context(tc.tile_pool(name="cvt", bufs=2))
                        if convert else None)
            pcm_pool = stream_sbuf_ctx.enter_context(tc.tile_pool(name="pcm", bufs=2))

            # ---------- stream + partial x + per-p RS ----------
            x_sb = [const.tile([M, B], rs_dt, name=f"x{p}") for p in range(P)]
            cc_in = [dram.tile([NCORES, M, NB], rs_dt, name=f"cci{p}") for p in range(P)]
            cc_out = [dram.tile([M, NB], rs_dt, name=f"cco{p}") for p in range(P)]
            cc_a2a = ([dram.tile([NCORES, M, NB], rs_dt, name=f"cca{p}")
                       for p in range(P)] if a2a else None)
            xt = [const.tile([M, NB], F32, name=f"xt{p}") for p in range(P)]
            sink = const.tile([128, 4], F32, name="sink", tag="sink") if ablate else None
            # packed q/k/v destinations, filled per-p as soon as xt[p] lands
            Q = const.tile([NB, P, H * K], F32, name="Q")
            K3 = const.tile([NB, P, H * K], F32, name="K3")
            V = const.tile([NB, P, H * K], F32, name="V")

            def qkv_phase(p, psum_pool, bufs):
                for w_sb, dst, bias in ((wq_sb, Q, bq_sb), (wk_sb, K3, bk_sb),
                                        (wv_sb, V, bv_sb)):
                    pp = psum_pool.tile([NB, H * K], F32, tag="sm", bufs=bufs)
                    nc.tensor.matmul(pp[:], lhsT=xt[p][:], rhs=w_sb[:],
                                     start=True, stop=True)
                    if dst is Q:
                        nc.vector.scalar_tensor_tensor(
                            dst[:, p, :], pp[:], INV_SQRT_K, bias[:],
                            ALU.mult, ALU.add)
                    else:
                        nc.vector.tensor_add(dst[:, p, :], pp[:], bias[:])

            def x_phase(p, ps):
                """psum pcmT -> fp16 -> transpose -> partial x -> RS -> xt."""
                pcmT = pcm_pool.tile([128, D], F16, tag="pcmT")
                nc.vector.tensor_copy(pcmT[:], ps[:])
                # transpose to (d part, m free)
                trp = trx_psum.tile([128, D], F16, tag="trx")
                for dt in range(NDT):
                    nc.tensor.transpose(trp[:, dt * 128:(dt + 1) * 128],
                                        pcmT[:, dt * 128:(dt + 1) * 128], ident_h[:])
                pcm_dd = pcm_pool.tile([128, D], F16, tag="pcmd")
                nc.vector.tensor_copy(pcm_dd[:], trp[:])
                # xT[p] (m part, b free) partial = sum_dt pcm_dd[dt].T @ poolsT[dt]
                xps = trx_psum.tile([128, B], F32, tag="trx2")
                for half in range(2):
                    for dt in range(NDT):
                        nc.tensor.matmul(
                            xps[:, half * 512:(half + 1) * 512],
                            lhsT=pcm_dd[:, dt * 128:(dt + 1) * 128],
                            rhs=poolsT_sb[:, dt, half * 512:(half + 1) * 512],
                            start=(dt == 0), stop=(dt == NDT - 1),
                        )
                nc.vector.tensor_copy(x_sb[p][:], xps[:])
                # reduce-scatter this p's x right away (overlaps later streams)
                nc.gpsimd.dma_start(
                    cc_in[p].rearrange("j m b -> m j b"),
                    x_sb[p][:].rearrange("m (j b) -> m j b", j=NCORES))
                if no_cc:
                    nc.gpsimd.dma_start(cc_out[p][:, :], cc_in[p][0, :, :])
                elif a2a:
                    nc.gpsimd.collective_compute(
                        "AllToAll", ALU.bypass,
                        replica_groups=[list(range(NCORES))],
                        ins=[cc_in[p][:, :, :]],
                        outs=[cc_a2a[p][:, :, :]],
                    )
                else:
                    nc.gpsimd.collective_compute(
                        "ReduceScatter", ALU.add,
                        replica_groups=[list(range(NCORES))],
                        ins=[cc_in[p][:, :, :]],
                        outs=[cc_out[p][:, :]],
                    )
                # load this core's xT slice back + rank-1 pc-quant correction
                if a2a and not no_cc:
                    # local 8-way sum of the gathered partials
                    x8 = const.tile([M, NCORES, NB], rs_dt, name=f"x8_{p}")
                    nc.gpsimd.dma_start(
                        x8[:], cc_a2a[p].rearrange("j m b -> m j b"))
                    x4 = const.tile([M, 4, NB], F32, name=f"x4_{p}")
                    nc.vector.tensor_add(x4[:], x8[:, 0:4, :], x8[:, 4:8, :])
                    x2 = const.tile([M, 2, NB], F32, name=f"x2_{p}")
                    nc.vector.tensor_add(x2[:], x4[:, 0:2, :], x4[:, 2:4, :])
                    xc = const.tile([M, NB], F32, name=f"xc_{p}")
                    nc.vector.tensor_add(xc[:], x2[:, 0, :], x2[:, 1, :])
                    nc.vector.tensor_add(xt[p][:], xc[:], corr_sb[p][:])
                else:
                    xth = const.tile([M, NB], rs_dt, name=f"xth{p}")
                    nc.gpsimd.dma_start(xth[:], cc_out[p][:, :])
                    nc.vector.tensor_add(xt[p][:], xth[:], corr_sb[p][:])

            pending = {}
            for p in range(P):
                ps = mm_psum.tile([128, D], F32, tag="mmps")
                for g in range(NG):
                    mp_t = mp_pool.tile([128, 4 * D], mpd, tag="mpt")
                    nc.sync.dma_start(mp_t[:], mp_d[p, g, :, :])
                    if convert:
                        mm_t = cvt_pool.tile([128, 4 * D], F16, tag="cvt")
                        # split upconvert across vector + scalar + gpsimd
                        nc.vector.tensor_copy(mm_t[:, 0:2 * D], mp_t[:, 0:2 * D])
                        nc.scalar.activation(mm_t[:, 2 * D:3 * D],
                                             mp_t[:, 2 * D:3 * D], AF.Copy)
                        nc.gpsimd.tensor_copy(mm_t[:, 3 * D:4 * D],
                                              mp_t[:, 3 * D:4 * D])
                    else:
                        mm_t = mp_t
                    for j in range(4):
                        blk = g * 4 + j
                        lw = pcT_sb[:, p, blk, :]
                        first = (blk == 0)
                        last = (blk == NCT - 1)
                        for ch in range(2):
                            nc.tensor.matmul(
                                ps[:, ch * 512:(ch + 1) * 512],
                                lhsT=lw,
                                rhs=mm_t[:, j * D + ch * 512:j * D + (ch + 1) * 512],
                                start=first, stop=last)
                if ablate == "nox":
                    nc.vector.tensor_copy(sink[:], ps[:, :4])
                    continue
                # software-pipeline: flush the previous p's x-phase so its PE
                # work lands between this p's and the next p's stream matmuls
                pending[p] = ps
                if p >= 1:
                    x_phase(p - 1, pending[p - 1])
            if ablate != "nox":
                x_phase(P - 1, pending[P - 1])

            # release stream-phase psum pools so the tail can use the banks
            stream_psum_ctx.close()
            stream_sbuf_ctx.close()

            if ablate is not None:
                if ablate == "notail":
                    for p in range(P):
                        nc.vector.tensor_copy(sink[:], xt[p][:, :4])
                nc.scalar.dma_start(out_d[:, :4], sink[:NB, :])
                continue

            # ---------- tail: attention on this core's batch shard ----------
            tail_ctx = ExitStack()
            tail = tail_ctx.enter_context(tc.tile_pool(name="tail", bufs=1))
            tail_ps = tail_ctx.enter_context(
                tc.tile_pool(name="tail_ps", bufs=1, space="PSUM"))

            # q/k/v: p0/p1's matmuls run during p2's RS wait; p2's gate scores
            for p in range(P):
                qkv_phase(p, tail_ps, 3)

            # scores S[b,p,q2,h] = sum_k Q[b,p,h,k]*K3[b,q2,h,k] (one mul+reduce)
            prod = tail.tile([NB, P, P, H, K], F32, name="prod")
            qv = Q[:].rearrange("b p (h k) -> b p h k", k=K)[:, :, None, :, :]
            kv = K3[:].rearrange("b p (h k) -> b p h k", k=K)[:, None, :, :, :]
            qb, kb = bass.broadcast_tensor_aps(qv, kv)
            nc.vector.tensor_tensor(prod[:], qb, kb, ALU.mult)
            S = tail.tile([NB, P, P, H], F32, name="S")
            nc.vector.tensor_reduce(
                S[:], prod[:], axis=mybir.AxisListType.X, op=ALU.add)

            # softmax over q2 (axis 2), batched over (p, h)
            mx = tail.tile([NB, P, H], F32, name="mx")
            nc.vector.tensor_reduce(
                mx[:], S[:].rearrange("b p q h -> b p h q"),
                axis=mybir.AxisListType.X, op=ALU.max)
            Et = tail.tile([NB, P, P, H], F32, name="Et")
            sb_, mb_ = bass.broadcast_tensor_aps(S[:], mx[:][:, :, None, :])
            nc.vector.tensor_tensor(Et[:], sb_, mb_, ALU.subtract)
            nc.scalar.activation(Et[:].rearrange("b p q h -> b (p q h)"),
                                 Et[:].rearrange("b p q h -> b (p q h)"), AF.Exp)
            den = tail.tile([NB, P, H], F32, name="den")
            nc.vector.tensor_reduce(
                den[:], Et[:].rearrange("b p q h -> b p h q"),
                axis=mybir.AxisListType.X, op=ALU.add)
            rden = tail.tile([NB, P, H], F32, name="rden")
            nc.vector.reciprocal(rden[:], den[:])
            ATT = tail.tile([NB, P, P, H], F32, name="ATT")
            eb_, rb_ = bass.broadcast_tensor_aps(Et[:], rden[:][:, :, None, :])
            nc.vector.tensor_tensor(ATT[:], eb_, rb_, ALU.mult)

            # O[b,p,h,k] = sum_q2 ATT[b,p,q2,h] * V[b,q2,h,k] (one mul+reduce)
            O5 = tail.tile([NB, P, P, H, K], F32, name="O5")
            ab_, vb_ = bass.broadcast_tensor_aps(
                ATT[:][:, :, :, :, None],
                V[:].rearrange("b q (h k) -> b q h k", k=K)[:, None, :, :, :])
            nc.vector.tensor_tensor(O5[:], ab_, vb_, ALU.mult)
            O = tail.tile([NB, P, H * K], F32, name="O")
            nc.vector.tensor_reduce(
                O[:].rearrange("b p (h k) -> b p h k", k=K),
                O5[:].rearrange("b p q h k -> b p h k q"),
                axis=mybir.AxisListType.X, op=ALU.add)

            # attendedT = sum_p wo.T @ O[p].T
            att_ps = tail_ps.tile([M, NB], F32, tag="attps")
            for p in range(P):
                oT_ps = tail_ps.tile([NB, H * K], F32, tag="sm", bufs=3)
                nc.tensor.transpose(oT_ps[:], O[:, p, :], ident_f[:])
                oT = tail.tile([H * K, NB], F32, tag="oT", bufs=3)
                nc.vector.tensor_copy(oT[:], oT_ps[:])
                nc.tensor.matmul(att_ps[:], lhsT=wo_sb[:], rhs=oT[:],
                                 start=(p == 0), stop=(p == P - 1))

            # preT (m, b) = attendedT summed over p, + 3*bo
            preT = tail.tile([M, NB], F32)
            nc.scalar.activation(preT[:], att_ps[:], AF.Identity, bias=bo3_sb[:])

            # pool_embeds (b, e) = preT.T @ pool_kernel + pool_bias
            pe_ps = tail_ps.tile([NB, E], F32, tag="sm", bufs=3)
            nc.tensor.matmul(pe_ps[:], lhsT=preT[:], rhs=pk_sb[:], start=True, stop=True)
            pe = tail.tile([NB, E], F32)
            nc.vector.tensor_add(pe[:], pe_ps[:], pb_sb[:])
            # l2 normalize along e
            sq2 = tail.tile([NB, E], F32)
            ss2 = tail.tile([NB, 1], F32)
            nc.scalar.activation(sq2[:], pe[:], AF.Square, accum_out=ss2[:])
            # 2*sqrt(ss + EPS) via Sqrt(4*ss + 4*EPS), so npn = pe/(2*||pe||)
            # and the augmented matmul below directly yields (cos+1)/2
            nc.scalar.activation(ss2[:], ss2[:], AF.Sqrt, bias=eps_sb[:NB, :],
                                 scale=4.0)
            rr = tail.tile([NB, 1], F32)
            nc.vector.reciprocal(rr[:], ss2[:])
            npn = tail.tile([NB, E], F32)
            nc.scalar.activation(npn[:], pe[:], AF.Copy, scale=rr[:])
            # transpose to (e, b), cast fp16 into npT rows 0:E (row E = 0.5)
            npT_ps = tail_ps.tile([E, NB], F32, tag="sm", bufs=3)
            nc.tensor.transpose(npT_ps[:], npn[:], ident_f[:])
            nc.vector.tensor_copy(npT[:E, :], npT_ps[:])

            # final: out = npT_aug.T @ ncT_aug = (cos+1)/2 directly; fp16
            # psum->sbuf copies rotate across vector/scalar/gpsimd, DMAs
            # alternate scalar/gpsimd queues
            for ch in range(C // 512):
                fp = tail_ps.tile([NB, 512], F32, tag="fin", bufs=4)
                nc.tensor.matmul(fp[:], lhsT=npT[:],
                                 rhs=ncT_sb[:, ch * 512:(ch + 1) * 512],
                                 start=True, stop=True)
                och = tail.tile([NB, 512], F16, tag="och", bufs=6)
                nc.vector.tensor_copy(och[:], fp[:])
                dq = (nc.scalar, nc.gpsimd)[ch % 2]
                dq.dma_start(out_d[:, ch * 512:(ch + 1) * 512], och[:])
            tail_ctx.close()

    _split_multi_waits(nc)
    return nc


def _silu64(x):
    return x / (1.0 + np.exp(-x))


def _prep_inputs(inputs, mp_dt=None):
    mp_dt = MP_DT if mp_dt is None else mp_dt
    mpnp = _MPNP[mp_dt]
    h16 = np.float16
    pools = np.asarray(inputs["pools"], np.float32)
    metapaths = np.asarray(inputs["metapaths"], np.float32)
    ce = np.asarray(inputs["card_embeddings"], np.float32)
    mpk = np.asarray(inputs["mp_kernels"], np.float32)
    mpb = np.asarray(inputs["mp_biases"], np.float32)

    # ---- host: pc[p] (C, M) in f64, fp16 cast + rank-1 quantization corr ----
    ce64 = ce.astype(np.float64)
    pc = np.empty((P, C, M), np.float64)
    for p in range(P):
        pc[p] = _silu64(ce64 @ mpk[p].astype(np.float64)
                        + mpb[p, :, 0].astype(np.float64)[None, :])
    pcT16 = np.empty((128, P, NCT, M), h16)   # [q, p, blk, m], c = blk*128+q
    for p in range(P):
        pcT16[:, p, :, :] = (pc[p].astype(np.float32)
                             .reshape(NCT, 128, M).transpose(1, 0, 2))
    # Sdelta[p][m] = sum_c (pc - fp16(pc))
    sdelta = np.empty((P, M), np.float64)
    for p in range(P):
        q16 = pcT16[:, p, :, :].transpose(1, 0, 2).reshape(C, M)
        sdelta[p] = (pc[p] - q16.astype(np.float64)).sum(axis=0)
    # w[p][b] = pools @ mean_c(mp[p]);  corr_x[p][m, b] = Sdelta[p][m] * w[p][b]
    mbar = metapaths.astype(np.float64).mean(axis=1)          # (P, C->d) wait axis
    # metapaths[p, c, d]: mean over c -> axis=1
    w = pools.astype(np.float64) @ mbar.transpose(1, 0)       # (B, P)
    corr_full = sdelta[:, :, None] * w.T[:, None, :]          # (P, M, B)

    # ---- host: mp stream repack (c = (g*4+j)*128 + q), per-core d shards ----
    mp_q = metapaths.astype(mpnp)   # (P, C, C)

    # ---- host: normalized cards (E, C) fp16 ----
    nrm = np.sqrt(np.maximum((ce.astype(np.float64) ** 2).sum(axis=1), EPS))
    ncT = (ce.astype(np.float64) / nrm[:, None]).T.astype(h16)  # (E, C)
    # augmented ones row: npT's 0.5 row x this row adds the +0.5 offset
    ncT = np.ascontiguousarray(np.concatenate(
        [ncT, np.ones((1, C), h16)], axis=0))

    poolsT = np.ascontiguousarray(pools.T).astype(h16)

    com = {
        "pcT16": np.ascontiguousarray(pcT16.reshape(128, NCT * P * M)),  # p-major
        "ncT": ncT,
        "wq": np.ascontiguousarray(np.asarray(inputs["Wq"], np.float32).reshape(M, H * K)),
        "wk": np.ascontiguousarray(np.asarray(inputs["Wk"], np.float32).reshape(M, H * K)),
        "wv": np.ascontiguousarray(np.asarray(inputs["Wv"], np.float32).reshape(M, H * K)),
        "wo": np.ascontiguousarray(np.asarray(inputs["Wo"], np.float32).reshape(H * K, M)),
        "bq_bc": np.ascontiguousarray(np.broadcast_to(
            np.asarray(inputs["bq"], np.float32).reshape(1, H * K), (NB, H * K))),
        "bk_bc": np.ascontiguousarray(np.broadcast_to(
            np.asarray(inputs["bk"], np.float32).reshape(1, H * K), (NB, H * K))),
        "bv_bc": np.ascontiguousarray(np.broadcast_to(
            np.asarray(inputs["bv"], np.float32).reshape(1, H * K), (NB, H * K))),
        "bo3_col": np.ascontiguousarray(
            (P * np.asarray(inputs["bo"], np.float32)).reshape(M, 1)),
        "pool_kernel": np.ascontiguousarray(np.asarray(inputs["pool_kernel"], np.float32)),
        "pool_bias_bc": np.ascontiguousarray(np.broadcast_to(
            np.asarray(inputs["pool_bias"], np.float32).reshape(1, E), (NB, E))),
        "ident_h": np.eye(128, dtype=h16),
        "ident_f": np.eye(128, dtype=np.float32),
    }
    in_maps = []
    for i in range(NCORES):
        m = dict(com)
        sl = mp_q[:, :, i * D:(i + 1) * D]                     # (P, C, D)
        st = sl.reshape(P, NG, 4, 128, D).transpose(0, 1, 3, 2, 4)
        m["mp_stream"] = np.ascontiguousarray(st.reshape(P, NG, 128, 4 * D))
        m["poolsT_shard"] = np.ascontiguousarray(poolsT[i * D:(i + 1) * D, :])
        m["corr"] = np.ascontiguousarray(
            corr_full[:, :, i * NB:(i + 1) * NB].astype(np.float32))
        in_maps.append(m)
    return in_maps


def kernel(**inputs) -> np.ndarray:
    if "nc" not in _CACHE:
        _CACHE["nc"] = build_kernel()
    nc = _CACHE["nc"]
    in_maps = _prep_inputs(inputs)
    res = run_bass_kernel_spmd(nc, in_maps, core_ids=list(range(NCORES)))
    outs = [np.asarray(res.results[i]["out"]).astype(np.float32)
            for i in range(NCORES)]
    return np.concatenate(outs, axis=0)


if __name__ == "__main__":
    nc = build_kernel()
    print("kernel built OK")



# revision 15
# speedup vs baseline: 1.1820x; 1.1820x over previous
"""Trainium2 Bass kernel for nn_MetapathRecommender, v3.

Shapes (hardcoded): B=1024, C=8192, P=3, E=64, M=128, H=16, K=8, 8 cores.

v3 changes vs v2 (284us -> target ~100us):
  - The dominant stream pcmT[p] = pc[p].T @ metapaths[p][:, dshard] runs as
    fp8e4m3 x fp8e4m3 with MatmulPerfMode.DoubleRow (2 c-rows per PE cell):
    half the matmul count at ~same per-matmul cost.  pc is host-quantized to
    e4m3 with a power-of-2 scale; the stream is repacked with the DoubleRow
    (Ki=128, Ko=2) interleave.  Stream becomes DMA-bound (~70us of fp8 HBM).
  - Each metapath's d-shard streams in TWO d-passes of 512 so the x-phase of
    pass0 overlaps the pass1 stream, halving the post-stream flush.
  - Rank-1 quantization corrections (pc-quant and mp-quant) are folded into
    the x matmul as two extra contraction rows (outer products), scaled by
    1/8 per core so the ReduceScatter sum reconstitutes them exactly.
  - x partials go psum -> (gpsimd cast DMA) -> collective input directly.
  - qkv is one packed matmul per metapath (Wq pre-scaled by 1/sqrt(K) on the
    host), issued as soon as that metapath's xt lands (overlaps the stream).
  - Attention tail restructured: per-metapath chains split across DVE and
    GpSimd, exp on the scalar engine; output head pipelined over 16 chunks
    with psum->sbuf copies rotating vector/scalar/gpsimd and the 2MB output
    DMA alternating the two HWDGE queues.
"""

import sys
from contextlib import ExitStack

sys.path.insert(0, "/opt/trn_rl_repo")

import numpy as np
import ml_dtypes

import concourse.bass as bass
import concourse.tile as tile
from concourse import mybir
from concourse.bass_utils import run_bass_kernel_spmd

F16 = mybir.dt.float16
F32 = mybir.dt.float32
F8E4 = mybir.dt.float8e4
AF = mybir.ActivationFunctionType
ALU = mybir.AluOpType
DR = mybir.MatmulPerfMode.DoubleRow

B, C, P, E, M, H, K = 1024, 8192, 3, 64, 128, 16, 8
HK = H * K
NCORES = 8
D = C // NCORES          # 1024: d-shard width per core
NB = B // NCORES         # 128: batch rows per core for the tail
NPASS = 2                # d-passes per metapath
DP = D // NPASS          # 512: d extent per pass
NG = 8                   # stream DMA groups per (p, pass); 4 c-pairs each
NPAIR = C // 256         # 32 DoubleRow c-block pairs
NDT = DP // 128          # 4 d tiles per pass
EPS = 1e-12
INV_SQRT_K = 1.0 / float(np.sqrt(np.float32(K)))
PC_SCALE = 256.0         # pc -> e4m3 scale (pow2; |pc|max*256 ~ 26 << 448)

_CACHE = {}


def _split_multi_waits(nc, cap=1):
    """Walrus in this container only accepts `cap` sync-waits per instruction.

    Move extra waits onto freshly inserted NoOps immediately preceding the
    instruction on the same engine."""
    f = nc.m.functions[0]
    nid = 0
    for blk in f.blocks:
        il = blk.instructions
        i = 0
        while i < len(il):
            inst = il[i]
            si = inst.sync_info
            if si is not None and len(si.on_wait) > cap:
                waits = list(si.on_wait)
                extra, keep = waits[:-cap], waits[-cap:]
                for w in extra:
                    nop = mybir.InstNoOp(
                        name=f"I-wsplit-{nid}", engine=inst.engine,
                        sync_info=mybir.SyncInfo(on_wait=[w], on_update=[]))
                    nid += 1
                    il.insert(i, nop)
                    i += 1
                inst.sync_info = mybir.SyncInfo(
                    on_wait=keep, on_update=list(si.on_update))
            i += 1
    return nid


def build_kernel(no_cc=False, repeat=1, ablate=None):
    nc = bass.Bass(num_devices=NCORES)

    # ---- kernel I/O (per-core shards / replicated small tensors) ----
    # mp stream, DoubleRow interleaved: [p, pass, g, ki, pair, ko, d]
    mp_d = nc.dram_tensor("mp_stream", [P, NPASS, NG, 128, 4 * 2 * DP], F8E4,
                          kind="ExternalInput")
    # pc weights, DoubleRow interleaved: [ki, p, pairblk, ko, m]
    pc8_d = nc.dram_tensor("pc8", [128, P * NPAIR * 2 * M], F8E4,
                           kind="ExternalInput")
    poolsT_d = nc.dram_tensor("poolsT_shard", [D, B], F16, kind="ExternalInput")
    # correction outer-product factors: lhs [p][2, M], rhs [p][2, B]
    corrL_d = nc.dram_tensor("corrL", [2 * P, M], F16, kind="ExternalInput")
    corrR_d = nc.dram_tensor("corrR", [2 * P, B], F16, kind="ExternalInput")
    ncT_d = nc.dram_tensor("ncT", [E + 1, C], F16, kind="ExternalInput")
    wqkv_d = nc.dram_tensor("wqkv", [M, 3 * HK], F16, kind="ExternalInput")
    bqkv_d = nc.dram_tensor("bqkv_bc", [NB, 3 * HK], F32, kind="ExternalInput")
    wo_d = nc.dram_tensor("wo", [HK, M], F32, kind="ExternalInput")
    bo3_d = nc.dram_tensor("bo3_col", [M, 1], F32, kind="ExternalInput")
    pk_d = nc.dram_tensor("pool_kernel", [M, E], F32, kind="ExternalInput")
    pb_d = nc.dram_tensor("pool_bias_bc", [NB, E], F32, kind="ExternalInput")
    ident_h_d = nc.dram_tensor("ident_h", [128, 128], F16, kind="ExternalInput")
    ident_f_d = nc.dram_tensor("ident_f", [128, 128], F32, kind="ExternalInput")

    out_d = nc.dram_tensor("out", [NB, C], F16, kind="ExternalOutput")

    with ExitStack() as ctx:
        tc = ctx.enter_context(tile.TileContext(nc, num_cores=NCORES))

        const = ctx.enter_context(tc.tile_pool(name="const", bufs=1))
        dram = ctx.enter_context(tc.tile_pool(name="dram", bufs=1, space="DRAM"))

        # ---------- load constants (stream-critical first, on scalar q) ----
        pc8_sb = const.tile([128, P, NPAIR, 2, M], F8E4, name="pc8")
        for p in range(P):
            nc.scalar.dma_start(
                pc8_sb[:, p, :, :, :].rearrange("q a b m -> q (a b m)"),
                pc8_d[:, p * NPAIR * 2 * M:(p + 1) * NPAIR * 2 * M])
        poolsT_sb = const.tile([128, D // 128, B], F16)  # (d%128, dtile, b)
        nc.scalar.dma_start(
            poolsT_sb[:], poolsT_d.ap().rearrange("(t p) b -> p t b", p=128))
        corrL_sb = [const.tile([2, M], F16, name=f"corrL{p}") for p in range(P)]
        corrR_sb = [const.tile([2, B], F16, name=f"corrR{p}") for p in range(P)]
        for p in range(P):
            nc.scalar.dma_start(corrL_sb[p][:], corrL_d[2 * p:2 * p + 2, :])
            nc.scalar.dma_start(corrR_sb[p][:], corrR_d[2 * p:2 * p + 2, :])
        wqkv_sb = const.tile([M, 3 * HK], F16)
        nc.scalar.dma_start(wqkv_sb[:], wqkv_d[:, :])
        bqkv_sb = const.tile([NB, 3 * HK], F32)
        nc.scalar.dma_start(bqkv_sb[:], bqkv_d[:, :])
        wo_sb = const.tile([HK, M], F32)
        nc.scalar.dma_start(wo_sb[:], wo_d[:, :])
        bo3_sb = const.tile([M, 1], F32)
        nc.scalar.dma_start(bo3_sb[:], bo3_d[:, :])
        pk_sb = const.tile([M, E], F32)
        nc.scalar.dma_start(pk_sb[:], pk_d[:, :])
        pb_sb = const.tile([NB, E], F32)
        nc.scalar.dma_start(pb_sb[:], pb_d[:, :])
        ident_h = const.tile([128, 128], F16)
        nc.scalar.dma_start(ident_h[:], ident_h_d[:, :])
        ident_f = const.tile([128, 128], F32)
        nc.scalar.dma_start(ident_f[:], ident_f_d[:, :])
        eps_sb = const.tile([128, 1], F32)
        nc.vector.memset(eps_sb[:], 4.0 * EPS)
        ncT_sb = const.tile([E + 1, C], F16)
        nc.scalar.dma_start(ncT_sb[:], ncT_d[:, :])
        # npT row E is the constant 0.5 that turns npT.T @ ncT_aug into
        # (cos + 1)/2 directly (ncT_aug's last row is all ones)
        npT = const.tile([E + 1, NB], F16, name="npT")
        nc.vector.memset(npT[E:E + 1, :], 0.5)

        for _rep in range(repeat):
            # ---------- stream + x partials + per-p RS + qkv ----------
            stream_ctx = ExitStack()
            mm_psum = stream_ctx.enter_context(
                tc.tile_pool(name="mm_psum", bufs=3, space="PSUM"))     # 3 banks
            trx_psum = stream_ctx.enter_context(
                tc.tile_pool(name="trx_psum", bufs=1, space="PSUM"))    # 1 bank
            xq_psum = stream_ctx.enter_context(
                tc.tile_pool(name="xq_psum", bufs=1, space="PSUM"))     # 2+1 banks
            mp_pool = stream_ctx.enter_context(tc.tile_pool(name="mp", bufs=12))
            pcm_pool = stream_ctx.enter_context(tc.tile_pool(name="pcm", bufs=4))

            cc_in = [dram.tile([NCORES, M, NB], F16, name=f"cci{p}") for p in range(P)]
            cc_out = [dram.tile([M, NB], F16, name=f"cco{p}") for p in range(P)]
            xt = [const.tile([M, NB], F16, name=f"xt{p}") for p in range(P)]
            QKV = const.tile([NB, P, 3 * HK], F32, name="QKV")
            sink = const.tile([128, 4], F16, name="sink", tag="sink") if ablate else None

            x_ps = {}

            def x_phase(p, pas, ps):
                """psum pcmT (pass) -> f16 -> transpose -> x partial MMs."""
                cp_eng = (nc.vector, nc.scalar)[pas]
                pcmT = pcm_pool.tile([128, DP], F16, tag="pcmT")
                if cp_eng is nc.scalar:
                    nc.scalar.activation(pcmT[:], ps[:], AF.Copy)
                else:
                    nc.vector.tensor_copy(pcmT[:], ps[:])
                trp = trx_psum.tile([128, DP], F16, tag="trx")
                for dt in range(NDT):
                    nc.tensor.transpose(trp[:, dt * 128:(dt + 1) * 128],
                                        pcmT[:, dt * 128:(dt + 1) * 128],
                                        ident_h[:])
                pcm_dd = pcm_pool.tile([128, DP], F16, tag="pcmd")
                if cp_eng is nc.scalar:
                    nc.vector.tensor_copy(pcm_dd[:], trp[:])
                else:
                    nc.scalar.activation(pcm_dd[:], trp[:], AF.Copy)
                xp = x_ps[p]
                for half in range(2):
                    hb = slice(half * 512, (half + 1) * 512)
                    for dt in range(NDT):
                        nc.tensor.matmul(
                            xp[:, hb],
                            lhsT=pcm_dd[:, dt * 128:(dt + 1) * 128],
                            rhs=poolsT_sb[:, pas * NDT + dt, hb],
                            start=(pas == 0 and dt == 0), stop=False)
                    if pas == NPASS - 1:
                        # rank-2 quantization correction as 2 extra rows
                        nc.tensor.matmul(
                            xp[:, hb],
                            lhsT=corrL_sb[p][:, :],
                            rhs=corrR_sb[p][:, hb],
                            start=False, stop=True)

            def finish_p(p):
                """x psum -> sbuf f16 -> cc_in -> RS -> xt -> packed qkv."""
                xp = x_ps[p]
                xsb = const.tile([M, B], F16, name=f"xsb{p}")
                nc.vector.tensor_copy(xsb[:, 0:512], xp[:, 0:512])
                nc.scalar.activation(xsb[:, 512:B], xp[:, 512:B], AF.Copy)
                nc.gpsimd.dma_start(
                    cc_in[p].rearrange("j m b -> m j b"),
                    xsb[:].rearrange("m (j b) -> m j b", j=NCORES))
                if no_cc:
                    nc.gpsimd.dma_start(cc_out[p][:, :], cc_in[p][0, :, :])
                else:
                    nc.gpsimd.collective_compute(
                        "ReduceScatter", ALU.add,
                        replica_groups=[list(range(NCORES))],
                        ins=[cc_in[p][:, :, :]],
                        outs=[cc_out[p][:, :]],
                    )
                nc.gpsimd.dma_start(xt[p][:], cc_out[p][:, :])
                if ablate == "notail":
                    nc.vector.tensor_copy(sink[:], xt[p][:, :4])
                    return
                qp = xq_psum.tile([NB, 3 * HK], F32, tag="qkv")
                nc.tensor.matmul(qp[:], lhsT=xt[p][:], rhs=wqkv_sb[:],
                                 start=True, stop=True)
                nc.vector.tensor_add(QKV[:, p, :], qp[:], bqkv_sb[:])

            # stream loop: p-pass pipeline with x_phase/finish interleaved
            pend = []
            for p in range(P):
                x_ps[p] = xq_psum.tile([M, B], F32, tag="xps", name="xps")
                for pas in range(NPASS):
                    ps = mm_psum.tile([128, DP], F32, tag="mmps")
                    for g in range(NG):
                        mp_t = mp_pool.tile([128, 4, 2, DP], F8E4, tag="mpt")
                        nc.sync.dma_start(
                            mp_t[:].rearrange("q a b d -> q (a b d)"),
                            mp_d[p, pas, g, :, :])
                        for pr in range(4):
                            pair = g * 4 + pr
                            nc.tensor.matmul(
                                ps[:],
                                lhsT=pc8_sb[:, p, pair, :, :],
                                rhs=mp_t[:, pr, :, :],
                                start=(pair == 0), stop=(pair == NPAIR - 1),
                                perf_mode=DR)
                    if ablate == "nox":
                        nc.vector.tensor_copy(sink[:], ps[:, :4])
                        continue
                    # finish_p(p-1) reads the shared x psum; it must be
                    # issued before x_phase(p, 0) starts overwriting it
                    if pas == NPASS - 1 and p >= 1:
                        finish_p(p - 1)
                    # flush previous pending x-phase AFTER issuing this
                    # pass's stream so PE work interleaves
                    pend.append((p, pas, ps))
                    if len(pend) >= 2:
                        x_phase(*pend.pop(0))
            if ablate != "nox":
                while pend:
                    x_phase(*pend.pop(0))
                finish_p(P - 1)

            stream_ctx.close()

            if ablate is not None:
                if ablate == "nox":
                    nc.gpsimd.dma_start(out_d[:, :4], sink[:NB, :])
                else:
                    nc.gpsimd.dma_start(out_d[:, :4], sink[:NB, :])
                continue

            # ---------- attention tail on this core's NB batch rows ----------
            # (small tiles live in `const` so DVE/GpSimd chains for p0/p1 can
            # run mid-stream without aliasing the stream pools)
            tail_ctx = ExitStack()
            tail = tail_ctx.enter_context(tc.tile_pool(name="tail", bufs=1))
            tail_ps = tail_ctx.enter_context(
                tc.tile_pool(name="tail_ps", bufs=1, space="PSUM"))

            Q = QKV[:].rearrange("b p (t h k) -> b p t h k", t=3, k=K)
            prod = const.tile([NB, P, H, P, K], F32, name="prod")
            S = const.tile([NB, P, H, P], F32, name="S")       # [b, p, h, q2]
            mx = const.tile([NB, P, H], F32, name="mx")
            Et = const.tile([NB, P, H, P], F32, name="Et")
            den = const.tile([NB, P, H], F32, name="den")
            rden = const.tile([NB, P, H], F32, name="rden")
            ATT = const.tile([NB, P, H, P], F32, name="ATT")
            prod2 = const.tile([NB, P, H, K, P], F32, name="prod2")
            O = const.tile([NB, P, HK], F32, name="O")

            # per-p serial chains on DVE (q2 innermost -> contiguous X
            # reduces); p0/p1 run mid-stream, only p2 is on the critical path
            for p in range(P):
                ve = nc.vector
                qv = Q[:, p, 0][:, :, None, :]                 # [b, h, 1, k]
                kv = Q[:, :, 1].rearrange("b q h k -> b h q k")
                qb, kb = bass.broadcast_tensor_aps(qv, kv)
                ve.tensor_tensor(prod[:, p], qb, kb, ALU.mult)
                ve.tensor_reduce(S[:, p], prod[:, p],
                                 axis=mybir.AxisListType.X, op=ALU.add)
                ve.tensor_reduce(mx[:, p], S[:, p],
                                 axis=mybir.AxisListType.X, op=ALU.max)
                sb_, mb_ = bass.broadcast_tensor_aps(S[:, p], mx[:, p][:, :, None])
                ve.tensor_tensor(Et[:, p], sb_, mb_, ALU.subtract)
                nc.scalar.activation(Et[:, p].rearrange("b h q -> b (h q)"),
                                     Et[:, p].rearrange("b h q -> b (h q)"),
                                     AF.Exp)
                ve.tensor_reduce(den[:, p], Et[:, p],
                                 axis=mybir.AxisListType.X, op=ALU.add)
                nc.vector.reciprocal(rden[:, p], den[:, p])
                eb_, rb_ = bass.broadcast_tensor_aps(Et[:, p], rden[:, p][:, :, None])
                ve.tensor_tensor(ATT[:, p], eb_, rb_, ALU.mult)
                # prod2[b, h, k, q2] = ATT[b, h, q2] * V[b, q2, h, k]
                av = ATT[:, p][:, :, None, :]                  # [b, h, 1, q2]
                vv = Q[:, :, 2].rearrange("b q h k -> b h k q")
                ab_, vb_ = bass.broadcast_tensor_aps(av, vv)
                ve.tensor_tensor(prod2[:, p], ab_, vb_, ALU.mult)
                ve.tensor_reduce(O[:, p].rearrange("b (h k) -> b h k", k=K),
                                 prod2[:, p],
                                 axis=mybir.AxisListType.X, op=ALU.add)

            # attendedT[m, b] = sum_p Wo.T @ O[p].T  (wo stationary)
            att_ps = tail_ps.tile([M, NB], F32, tag="attps")
            oT = [const.tile([HK, NB], F32, name=f"oT{p}") for p in range(P)]
            for p in range(P):
                oT_ps = tail_ps.tile([NB, HK], F32, tag="otps", bufs=2)
                nc.tensor.transpose(oT_ps[:], O[:, p, :], ident_f[:])
                if p % 2:
                    nc.scalar.activation(oT[p][:], oT_ps[:], AF.Copy)
                else:
                    nc.vector.tensor_copy(oT[p][:], oT_ps[:])
            for p in range(P):
                nc.tensor.matmul(att_ps[:], lhsT=wo_sb[:], rhs=oT[p][:],
                                 start=(p == 0), stop=(p == P - 1))

            # preT (m, b) = attendedT + 3*bo ; pool head
            preT = const.tile([M, NB], F32, name="preT")
            nc.scalar.activation(preT[:], att_ps[:], AF.Identity, bias=bo3_sb[:])
            pe_ps = tail_ps.tile([NB, E], F32, tag="peps")
            nc.tensor.matmul(pe_ps[:], lhsT=preT[:], rhs=pk_sb[:],
                             start=True, stop=True)
            pe = const.tile([NB, E], F32, name="pe")
            nc.vector.tensor_add(pe[:], pe_ps[:], pb_sb[:])
            sq2 = const.tile([NB, E], F32, name="sq2")
            ss2 = const.tile([NB, 1], F32, name="ss2")
            nc.scalar.activation(sq2[:], pe[:], AF.Square, accum_out=ss2[:])
            # 2*sqrt(ss + EPS) via Sqrt(4*ss + 4*EPS), so npn = pe/(2*||pe||)
            nc.scalar.activation(ss2[:], ss2[:], AF.Sqrt, bias=eps_sb[:NB, :],
                                 scale=4.0)
            rr = const.tile([NB, 1], F32, name="rr")
            nc.vector.reciprocal(rr[:], ss2[:])
            npn = const.tile([NB, E], F32, name="npn")
            nc.scalar.activation(npn[:], pe[:], AF.Copy, scale=rr[:])
            npT_ps = tail_ps.tile([E, NB], F32, tag="npps")
            nc.tensor.transpose(npT_ps[:], npn[:], ident_f[:])
            nc.vector.tensor_copy(npT[:E, :], npT_ps[:])

            # final: out = npT_aug.T @ ncT_aug = (cos+1)/2 directly
            dq_engs = (nc.scalar, nc.sync, nc.gpsimd)
            for ch in range(C // 512):
                fp = tail_ps.tile([NB, 512], F32, tag="fin", bufs=3)
                nc.tensor.matmul(fp[:], lhsT=npT[:],
                                 rhs=ncT_sb[:, ch * 512:(ch + 1) * 512],
                                 start=True, stop=True)
                och = tail.tile([NB, 512], F16, tag="och", bufs=6)
                if ch % 2:
                    nc.scalar.activation(och[:], fp[:], AF.Copy)
                else:
                    nc.vector.tensor_copy(och[:], fp[:])
                dq_engs[ch % 3].dma_start(out_d[:, ch * 512:(ch + 1) * 512],
                                          och[:])
            tail_ctx.close()

    _split_multi_waits(nc)
    return nc


def _silu64(x):
    return x / (1.0 + np.exp(-x))


def _prep_inputs(inputs):
    f8 = ml_dtypes.float8_e4m3
    h16 = np.float16
    pools = np.asarray(inputs["pools"], np.float32)
    metapaths = np.asarray(inputs["metapaths"], np.float32)
    ce = np.asarray(inputs["card_embeddings"], np.float32)
    mpk = np.asarray(inputs["mp_kernels"], np.float32)
    mpb = np.asarray(inputs["mp_biases"], np.float32)

    # ---- host: pc[p] (C, M) in f64 -> e4m3 (scaled) + DoubleRow repack ----
    ce64 = ce.astype(np.float64)
    pc = np.empty((P, C, M), np.float64)
    for p in range(P):
        pc[p] = _silu64(ce64 @ mpk[p].astype(np.float64)
                        + mpb[p, :, 0].astype(np.float64)[None, :])
    pc8 = (pc * PC_SCALE).astype(f8)                      # (P, C, M)
    pcq = pc8.astype(np.float64) / PC_SCALE
    # pc8 repack: [ki, p, pair, ko, m], c = (2*pair + ko)*128 + ki
    pc8_r = (pc8.reshape(P, NPAIR, 2, 128, M)
             .transpose(3, 0, 1, 2, 4))                   # ki, p, pair, ko, m
    pc8_flat = np.ascontiguousarray(pc8_r.reshape(128, P * NPAIR * 2 * M))

    # ---- host: mp -> e4m3 ----
    mp8 = metapaths.astype(f8)                            # (P, C, C)
    mpq = mp8.astype(np.float64)

    # ---- rank-1 corrections (pc-quant and mp-quant), as outer products ----
    # x_err = sum_c dpc[c,m] * z[b,c] + sum_c pcq[c,m] * dz[b,c]
    #   dpc approx: Sdelta[m] * w[b],  w = pools @ mean_c(mp)
    #   dmp approx: u[m] * w2[b],      u = sum_c pcq,  w2 = pools @ mean_c(dmp)
    corrL = np.empty((2 * P, M), np.float64)
    corrR = np.empty((2 * P, B), np.float64)
    pools64 = pools.astype(np.float64)
    for p in range(P):
        sdelta = (pc[p] - pcq[p]).sum(axis=0)             # (M,)
        w = pools64 @ metapaths[p].astype(np.float64).mean(axis=0)   # (B,)
        u = pcq[p].sum(axis=0)                            # (M,)
        w2 = pools64 @ (metapaths[p].astype(np.float64) - mpq[p]).mean(axis=0)
        # balance the factor magnitudes for f16 storage
        for row, (lv, rv) in enumerate(((sdelta, w), (u, w2))):
            s = np.sqrt((np.abs(lv).max() + 1e-30) / (np.abs(rv).max() + 1e-30))
            corrL[2 * p + row] = lv / s
            corrR[2 * p + row] = rv * s
    corrR /= NCORES   # each core contributes corr/8; RS sums to corr

    # ---- host: normalized cards (E+1, C) fp16, ones row appended ----
    nrm = np.sqrt(np.maximum((ce.astype(np.float64) ** 2).sum(axis=1), EPS))
    ncT = (ce.astype(np.float64) / nrm[:, None]).T.astype(h16)
    ncT = np.ascontiguousarray(np.concatenate(
        [ncT, np.ones((1, C), h16)], axis=0))

    # 1/PC_SCALE folded into poolsT: x = (pc*256).T @ mp @ (pools/256)
    poolsT = np.ascontiguousarray(pools.T / PC_SCALE).astype(h16)

    # ---- packed qkv weights; 1/sqrt(K) folded into Wq (and bq) ----
    wq = np.asarray(inputs["Wq"], np.float32).reshape(M, HK) * INV_SQRT_K
    wk = np.asarray(inputs["Wk"], np.float32).reshape(M, HK)
    wv = np.asarray(inputs["Wv"], np.float32).reshape(M, HK)
    wqkv = np.concatenate([wq, wk, wv], axis=1)           # (M, 384)
    bq = np.asarray(inputs["bq"], np.float32).reshape(HK) * INV_SQRT_K
    bk = np.asarray(inputs["bk"], np.float32).reshape(HK)
    bv = np.asarray(inputs["bv"], np.float32).reshape(HK)
    bqkv = np.concatenate([bq, bk, bv])

    com = {
        "pc8": pc8_flat,
        "corrL": np.ascontiguousarray(corrL.astype(h16)),
        "ncT": ncT,
        "wqkv": np.ascontiguousarray(wqkv.astype(h16)),
        "bqkv_bc": np.ascontiguousarray(np.broadcast_to(
            bqkv.reshape(1, 3 * HK), (NB, 3 * HK)).astype(np.float32)),
        "wo": np.ascontiguousarray(np.asarray(inputs["Wo"], np.float32).reshape(HK, M)),
        "bo3_col": np.ascontiguousarray(
            (P * np.asarray(inputs["bo"], np.float32)).reshape(M, 1)),
        "pool_kernel": np.ascontiguousarray(np.asarray(inputs["pool_kernel"], np.float32)),
        "pool_bias_bc": np.ascontiguousarray(np.broadcast_to(
            np.asarray(inputs["pool_bias"], np.float32).reshape(1, E), (NB, E))),
        "ident_h": np.eye(128, dtype=h16),
        "ident_f": np.eye(128, dtype=np.float32),
    }
    in_maps = []
    for i in range(NCORES):
        m = dict(com)
        # mp stream repack: [p, pass, g, ki, pair, ko, d]
        # c = (2*(g*4+pair) + ko)*128 + ki ; d = i*D + pass*DP + dlocal
        sl = mp8[:, :, i * D:(i + 1) * D]                 # (P, C, D)
        st = sl.reshape(P, NG, 4, 2, 128, NPASS, DP)      # c-major split
        st = st.transpose(0, 5, 1, 4, 2, 3, 6)            # p, pass, g, ki, pair, ko, d
        m["mp_stream"] = np.ascontiguousarray(
            st.reshape(P, NPASS, NG, 128, 4 * 2 * DP))
        m["poolsT_shard"] = np.ascontiguousarray(poolsT[i * D:(i + 1) * D, :])
        m["corrR"] = np.ascontiguousarray(corrR.astype(h16))
        in_maps.append(m)
    return in_maps


def kernel(**inputs) -> np.ndarray:
    if "nc" not in _CACHE:
        _CACHE["nc"] = build_kernel()
    nc = _CACHE["nc"]
    in_maps = _prep_inputs(inputs)
    res = run_bass_kernel_spmd(nc, in_maps, core_ids=list(range(NCORES)))
    outs = [np.asarray(res.results[i]["out"]).astype(np.float32)
            for i in range(NCORES)]
    return np.concatenate(outs, axis=0)


if __name__ == "__main__":
    nc = build_kernel()
    print("kernel built OK")
